# revision 35
# baseline (speedup 1.0000x reference)
"""Trainium2 Bass kernel for DifferentiableExtrusion.

Full inputs in, full output out. Sharding: the 96x96=9216 grid points are
split across 8 cores (12 grid rows / 1152 points each). Every core processes
all valid polygons (host-compacted, variable count per batch) against its
points:

  per (point, edge):  d^2 = l^2 + r^2   with
      l = v . n_hat               (line distance, affine in point -> PE matmul)
      u = v . e / sqrt(e^2+eps)   (affine in point -> PE matmul)
      r = u - clip(u, 0, S)       (projection excess)
  inside test: ray-cast parity of [(sign(py-y0) != sign(py-y1)) & (G > 0)]
      with G = inter_x - px       (affine in point -> PE matmul)
  The y-comparisons depend only on the point's grid row: computed once per
  core at [12, E] and DMA-broadcast across partitions per chunk.
  sdf = sign * sqrt(min_edges d^2); per-batch min over polys taken on
  sign*(d^2) (order-equivalent); sqrt+sigmoid deferred to one end stage so
  the ACT engine stays on a single function table during the main loop.
  Depth extrusion = K=1 outer-product matmul with the depth mask.

Each core writes out[b, d, its 12 rows] = [4, 96, 1152]; host concatenates.
"""

import numpy as np

VOX = 96
SHARP = 100.0
EPS = 1e-8
NCORES = 8
M = VOX * VOX
MP = M // NCORES          # 1152 points per core
CHUNKS = MP // 128        # 9
ROWS = MP // VOX          # 12 grid rows per core
PEDGES = 32               # edges per polygon
BIGD = 1e3                # far-outside distance for dummy (empty-batch) polys

# The affine tables/features are built from bf16-exact split components
# (hi+lo), so plain bf16 matmuls with K=8 reconstruct fp32-grade products at
# full PE speed.
MM_DTYPE = "bfloat16"


def _host_prep(polygons, attributes, validity_scores):
    B, N, P, _ = polygons.shape
    assert P == PEDGES
    valid = np.asarray(validity_scores) >= 0.5
    counts = [max(1, int(v.sum())) for v in valid]   # >=1: empty batch gets a dummy
    offs = np.cumsum([0] + counts)
    NPT = int(offs[-1])
    E = NPT * P

    v0 = np.asarray(polygons, np.float32).astype(np.float64)
    v1 = np.roll(v0, -1, axis=2)
    x0, y0 = v0[..., 0], v0[..., 1]
    x1, y1 = v1[..., 0], v1[..., 1]
    ex, ey = x1 - x0, y1 - y0
    esq = ex * ex + ey * ey
    esq_c = np.maximum(esq, 1e-12)
    Sp = np.sqrt(esq + EPS)
    rt = np.sqrt(esq_c)
    s = ex / (ey + EPS)

    cu = np.stack([ex / Sp, ey / Sp, -(x0 * ex + y0 * ey) / Sp], -1)
    cl = np.stack([-ey / rt, ex / rt, (ey * x0 - ex * y0) / rt], -1)
    cg = np.stack([-np.ones_like(s), s, x0 - s * y0], -1)

    wu = np.zeros((3, E), np.float32)
    wl = np.zeros((3, E), np.float32)
    wg = np.zeros((3, E), np.float32)
    y0r = np.full(E, 5.0, np.float32)
    y1r = np.full(E, 5.0, np.float32)
    sr = np.ones(E, np.float32)
    wl[2, :] = BIGD          # dummy cols: u=0, l=BIGD, G=-1 -> far outside
    wg[2, :] = -1.0

    for b in range(B):
        idx = np.nonzero(valid[b])[0]
        for k, n in enumerate(idx):
            c0 = (offs[b] + k) * P
            sl = slice(c0, c0 + P)
            wu[:, sl] = cu[b, n].T.astype(np.float32)
            wl[:, sl] = cl[b, n].T.astype(np.float32)
            wg[:, sl] = cg[b, n].T.astype(np.float32)
            y0r[sl] = y0[b, n].astype(np.float32)
            y1r[sl] = y1[b, n].astype(np.float32)
            sr[sl] = Sp[b, n].astype(np.float32)

    # split-precision expansion: value = hi + lo with bf16-exact components, so
    # fp32r products are computed exactly by the PE regardless of its internal
    # mantissa truncation (>= 8 bits).  features [hx, lx, hx, hy, ly, hy, 1, 1]
    # against weights [w_hi, w_hi, w_lo, ...] reconstruct px*w to ~2^-16.
    import ml_dtypes

    def b16split(x):
        hi = x.astype(ml_dtypes.bfloat16).astype(np.float32)
        lo = (x - hi).astype(ml_dtypes.bfloat16).astype(np.float32)
        return hi, lo

    def expand_w(w):
        out = np.zeros((8, w.shape[1]), np.float32)
        for i in range(3):
            hi, lo = b16split(w[i])
            j = i * 3 if i < 2 else 6
            if i < 2:
                out[j] = hi; out[j + 1] = hi; out[j + 2] = lo
            else:
                out[6] = hi; out[7] = lo
        return out

    wu = expand_w(wu).astype(ml_dtypes.bfloat16)
    wl = expand_w(wl).astype(ml_dtypes.bfloat16)
    wg = expand_w(wg).astype(ml_dtypes.bfloat16)

    ygrid, xgrid = np.meshgrid(np.arange(VOX, dtype=np.float32),
                               np.arange(VOX, dtype=np.float32), indexing="ij")
    px = (xgrid.ravel() / np.float32(VOX - 1)).astype(np.float32)
    py = (ygrid.ravel() / np.float32(VOX - 1)).astype(np.float32)

    feats, ysqbs, cntbs = [], [], []
    for k in range(NCORES):
        sl = slice(k * MP, (k + 1) * MP)
        hx, lx = b16split(px[sl])
        hy, ly = b16split(py[sl])
        one = np.ones(MP, np.float32)
        f = np.stack([hx, lx, hx, hy, ly, hy, one, one], 0)
        feats.append(np.ascontiguousarray(f.astype(ml_dtypes.bfloat16)))
        # ysq[row, e] = (y0<=py) xor (y1<=py), per grid row of this core,
        # expanded to the per-chunk partition->row broadcast pattern
        rows = (np.arange(ROWS, dtype=np.float32) + k * ROWS) / np.float32(VOX - 1)
        t0c = (y0r[None, :] <= rows[:, None])
        t1c = (y1r[None, :] <= rows[:, None])
        ysq12 = (t0c ^ t1c).astype(np.float32)            # [ROWS, E]
        rowidx = (np.arange(MP) // VOX).astype(np.int64)  # local row per point
        ysqbs.append(np.ascontiguousarray(
            ysq12[rowidx].reshape(CHUNKS, 128, E).astype(ml_dtypes.bfloat16)))
        # per-(row, poly) active-edge counts: cnt = sum ysq*sign(G) + cntb
        cb12 = ysq12.reshape(ROWS, NPT, PEDGES).sum(2)    # [ROWS, NPT]
        cntbs.append(np.ascontiguousarray(
            cb12[rowidx].reshape(CHUNKS, 128, NPT).astype(np.int32)))

    attr = np.asarray(attributes, np.float32)
    norm_h = np.clip(attr[:, 0], 0.0, 1.0)
    hv = np.clip(np.round(norm_h * VOX), 1.0, float(VOX)).astype(np.float32)
    hvs = [0 if not valid[b].any() else int(hv[b]) for b in range(B)]

    tables = {
        "wu": wu, "wl": wl, "wg": wg,
        "sbc": np.ascontiguousarray(np.ones((128, 1), np.float32) * sr[None, :]),
        "ident": np.eye(128, dtype=np.float32),
    }
    return tables, feats, ysqbs, cntbs, counts, E, hvs


def _blocks(E):
    nblk = (E + 511) // 512
    per = -(-E // (32 * nblk)) * 32           # even-ish blocks, multiple of 32
    out = []
    o = 0
    while o < E:
        nb = min(per, E - o)
        out.append((o, nb))
        o += nb
    return out


def _build(B, counts, E, hvs):
    import concourse.tile as tile
    from concourse import bacc, mybir

    f32 = mybir.dt.float32
    i32 = mybir.dt.int32
    bf16 = mybir.dt.bfloat16
    mmdt = getattr(mybir.dt, MM_DTYPE)

    Op = mybir.AluOpType
    Act = mybir.ActivationFunctionType
    X = mybir.AxisListType.X
    NPT = sum(counts)
    offs = np.cumsum([0] + list(counts))
    blocks = _blocks(E)

    nc = bacc.Bacc("TRN2", target_bir_lowering=False, debug=False)

    din = {}
    for name, shape in [("wu", [8, E]), ("wl", [8, E]), ("wg", [8, E]),
                        ("sbc", [128, E]), ("feat", [8, MP]),
                        ("ysqb_all", [CHUNKS, 128, E]),
                        ("cntb_all", [CHUNKS, 128, NPT]),
                        ("ident", [128, 128])]:
        if name in ("wu", "wl", "wg", "feat", "ysqb_all"):
            dt = mmdt
        elif name == "cntb_all":
            dt = mybir.dt.int32
        else:
            dt = f32
        din[name] = nc.dram_tensor(name, shape, dt, kind="ExternalInput")
    out_d = nc.dram_tensor("out", [B, VOX, MP], f32, kind="ExternalOutput")
    comb_d = nc.dram_tensor("comb_scratch", [B, 4, MP], f32)

    with tile.TileContext(nc) as tc:
        with tc.tile_pool(name="const", bufs=1) as cpool, \
             tc.tile_pool(name="work", bufs=4) as wpool, \
             tc.tile_pool(name="ybc", bufs=2) as ypool, \
             tc.tile_pool(name="acc", bufs=2) as apool, \
             tc.tile_pool(name="psum3", bufs=3, space="PSUM") as ppool3, \
             tc.tile_pool(name="psum", bufs=2, space="PSUM") as ppool, \
             tc.tile_pool(name="pout", bufs=1, space="PSUM") as opool:

            feat = cpool.tile([8, MP], mmdt)
            nc.sync.dma_start(feat[:], din["feat"][:])
            sb = {}
            for name in ["wu", "wl", "wg"]:
                t = cpool.tile([8, E], mmdt, tag=f"c_{name}", name=f"c_{name}")
                nc.sync.dma_start(t[:], din[name][:])
                sb[name] = t
            sbc = cpool.tile([128, E], f32)
            for i, (j0, nb) in enumerate(blocks):
                eng = nc.sync if i == 0 else nc.scalar
                eng.dma_start(sbc[:, j0:j0 + nb], din["sbc"][:, j0:j0 + nb])
            ident = cpool.tile([128, 128], f32)
            nc.scalar.dma_start(ident[:], din["ident"][:])
            cntb = cpool.tile([128, CHUNKS, NPT], i32)
            for c in range(CHUNKS):
                nc.scalar.dma_start(cntb[:, c, :], din["cntb_all"][c])

            qall = cpool.tile([128, B * 32], f32)
            nc.gpsimd.memset(qall[:], 0)
            qbig = cpool.tile([128, CHUNKS, NPT], f32)
            comb = []
            for b in range(B):
                comb_b = cpool.tile([CHUNKS, 128], f32, tag=f"comb{b}",
                                    name=f"comb{b}")
                comb.append(comb_b)

            warm = cpool.tile([1, 1], f32)
            nc.gpsimd.memset(warm[:], 0)
            deferred = []

            def run_deferred(keep):
                while len(deferred) > keep:
                    deferred.pop(0)()

            for c in range(CHUNKS):
                featc = feat[:, c * 128:(c + 1) * 128]
                ysqb = ypool.tile([128, E], bf16, tag="ysqb", name="ysqb")
                if c == 0:
                    for i, (j0, nb) in enumerate(blocks):
                        eng = nc.sync if i == 0 else nc.scalar
                        eng.dma_start(ysqb[:, j0:j0 + nb],
                                      din["ysqb_all"][c][:, j0:j0 + nb])
                else:
                    nc.sync.dma_start(ysqb[:], din["ysqb_all"][c])

                mind2 = apool.tile([128, NPT], f32, tag="mind2")
                cnt = apool.tile([128, NPT], i32, tag="cnt")

                def reduces(j0, npj, d2, cross, mind2=None, cnt=None):
                    pj = slice(j0 // PEDGES, j0 // PEDGES + npj)
                    nc.vector.tensor_reduce(
                        mind2[:, pj],
                        d2[:].rearrange("p (a b) -> p a b", b=PEDGES),
                        axis=X, op=Op.min)
                    with nc.allow_low_precision(
                            reason="crossing counts are small exact ints"):
                        nc.vector.tensor_reduce(
                            cnt[:, pj],
                            cross[:].rearrange("p (a b) -> p a b", b=PEDGES),
                            axis=X, op=Op.add)

                for j0, nb in blocks:
                    npj = nb // PEDGES
                    cols = slice(j0, j0 + nb)
                    ups = ppool3.tile([128, nb], f32, tag="u")
                    lps = ppool.tile([128, nb], f32, tag="l")
                    gps = ppool.tile([128, nb], f32, tag="g")
                    nc.tensor.matmul(ups[:], featc, sb["wu"][:, cols])
                    nc.tensor.matmul(lps[:], featc, sb["wl"][:, cols])
                    nc.tensor.matmul(gps[:], featc, sb["wg"][:, cols])

                    m = wpool.tile([128, nb], f32, tag="m")
                    nc.vector.scalar_tensor_tensor(
                        m[:], ups[:], 0.0, sbc[:, cols], op0=Op.max, op1=Op.min)
                    r = wpool.tile([128, nb], f32, tag="r")
                    nc.vector.tensor_tensor(r[:], ups[:], m[:], op=Op.subtract)
                    lsq = wpool.tile([128, nb], f32, tag="lsq")
                    nc.scalar.activation(lsq[:], lps[:], Act.Square)
                    rsq = wpool.tile([128, nb], f32, tag="rsq")
                    nc.scalar.activation(rsq[:], r[:], Act.Square)
                    d2 = wpool.tile([128, nb], f32, tag="d2")
                    nc.gpsimd.tensor_tensor(d2[:], lsq[:], rsq[:], op=Op.add)

                    gs = wpool.tile([128, nb], bf16, tag="gs")
                    nc.scalar.activation(gs[:], gps[:], Act.Sign)
                    cross = wpool.tile([128, nb], bf16, tag="cross")
                    nc.gpsimd.tensor_tensor(cross[:], gs[:], ysqb[:, cols],
                                            op=Op.mult)

                    deferred.append(
                        lambda a=j0, b_=npj, d=d2, x=cross, mi=mind2, cn=cnt:
                        reduces(a, b_, d, x, mi, cn))
                    run_deferred(3)

                def chunk_tail(c=c, mind2=mind2, cnt=cnt):
                    # cnt + cntb = 2*crossings (exact ints); parity from bit 1
                    odd2 = wpool.tile([128, NPT], i32, tag="odd2")
                    cnt2 = wpool.tile([128, NPT], i32, tag="cnt2")
                    nc.vector.tensor_tensor(cnt2[:], cnt[:], cntb[:, c, :],
                                            op=Op.add)
                    nc.vector.tensor_scalar(odd2[:], cnt2[:], 2, None,
                                            op0=Op.bitwise_and)
                    sgn = wpool.tile([128, NPT], f32, tag="sgn")
                    nc.vector.tensor_scalar(sgn[:], odd2[:], -1.0, 1.0,
                                            op0=Op.mult, op1=Op.add)
                    nc.vector.tensor_tensor(qbig[:, c, :], mind2[:], sgn[:],
                                            op=Op.mult)

                deferred.append(chunk_tail)
                if c == CHUNKS - 2:
                    nc.scalar.activation(warm[:], warm[:], Act.Sqrt)
            run_deferred(0)

            # per-batch min over polys, all chunks at once (writes the
            # transpose-ready [128, 32b+c] layout)
            for b in range(B):
                nc.vector.tensor_reduce(
                    qall[:, 32 * b:32 * b + CHUNKS],
                    qbig[:, :, offs[b]:offs[b + 1]], axis=X, op=Op.min)

            # end stage: sdf = sign(q)*sqrt(|q|), one sigmoid + one transpose;
            # after the transpose, batch b's 9 chunk-rows sit at partitions
            # 32b..32b+8 (aligned base for the per-batch copies)
            absq = wpool.tile([128, B * 32], f32, tag="absq")
            nc.scalar.activation(absq[:], qall[:], Act.Abs)
            dst = wpool.tile([128, B * 32], f32, tag="dst")
            nc.scalar.activation(dst[:], absq[:], Act.Sqrt)
            sgq = wpool.tile([128, B * 32], f32, tag="sgq")
            nc.scalar.activation(sgq[:], qall[:], Act.Sign)
            sdf = wpool.tile([128, B * 32], f32, tag="sdf")
            nc.vector.tensor_tensor(sdf[:], dst[:], sgq[:], op=Op.mult)
            cpb = wpool.tile([128, B * 32], f32, tag="cpb")
            nc.scalar.activation(cpb[:], sdf[:], Act.Sigmoid, scale=-SHARP)
            pst = opool.tile([128, 128], f32, tag="pp", name="pst")
            nc.tensor.transpose(pst[:], cpb[:], ident[:])
            for b in range(B):
                nc.scalar.activation(comb[b][:], pst[32 * b:32 * b + CHUNKS, :],
                                     Act.Copy)

            # depth extrusion: replicate combined[b] into rows [0, hv_b) with
            # independent parallel broadcast DMAs (16-row groups) from a DRAM
            # bounce row; rows >= hv_b stay zero (outputs are donated zero
            # buffers).  Dispatch spread over the three DMA-capable engines.
            engs = [nc.sync, nc.gpsimd, nc.scalar]
            ei = 0
            for b in range(B):
                if hvs[b] == 0:
                    continue
                for rep in range(4):
                    engs[ei % 3].dma_start(comb_d[b, rep:rep + 1, :], comb[b][:])
                    ei += 1
            GRP = 16
            for b in range(B):
                g0 = 0
                while g0 + 4 <= hvs[b]:
                    n = min(GRP, (hvs[b] - g0) // 4 * 4)
                    engs[ei % 3].dma_start(
                        out_d[b, g0:g0 + n, :],
                        comb_d[b:b + 1, :, :].partition_broadcast(n // 4))
                    ei += 1
                    g0 += n
                if g0 < hvs[b]:
                    engs[ei % 3].dma_start(
                        out_d[b, g0:hvs[b], :],
                        comb_d[b, 0:1, :].partition_broadcast(hvs[b] - g0))
                    ei += 1

    nc.compile()
    return nc


def kernel(polygons, attributes, validity_scores):
    from concourse.bass_utils import run_bass_kernel_spmd

    B = polygons.shape[0]
    tables, feats, ysqbs, cntbs, counts, E, hvs = _host_prep(
        polygons, attributes, validity_scores)
    nc = _build(B, counts, E, hvs)
    in_maps = [dict(tables, feat=feats[k], ysqb_all=ysqbs[k], cntb_all=cntbs[k])
               for k in range(NCORES)]
    res = run_bass_kernel_spmd(nc, in_maps, list(range(NCORES))).results
    parts = [res[k]["out"].reshape(B, VOX, VOX // NCORES, VOX)
             for k in range(NCORES)]
    return np.ascontiguousarray(np.concatenate(parts, axis=2), np.float32)


# revision 36
# speedup vs baseline: 1.0618x; 1.0618x over previous
"""Trainium2 Bass kernel for DifferentiableExtrusion.

Full inputs in, full output out. Sharding: the 96x96=9216 grid points are
split across 8 cores (12 grid rows / 1152 points each). Every core processes
all valid polygons (host-compacted, variable count per batch) against its
points:

  per (point, edge):  d^2 = l^2 + r^2   with
      l = v . n_hat               (line distance, affine in point -> PE matmul)
      u = v . e / sqrt(e^2+eps)   (affine in point -> PE matmul)
      r = u - clip(u, 0, S)       (projection excess)
  inside test: ray-cast parity of [(sign(py-y0) != sign(py-y1)) & (G > 0)]
      with G = inter_x - px       (affine in point -> PE matmul)
  The y-comparisons depend only on the point's grid row: computed once per
  core at [12, E] and DMA-broadcast across partitions per chunk.
  sdf = sign * sqrt(min_edges d^2); per-batch min over polys taken on
  sign*(d^2) (order-equivalent); sqrt+sigmoid deferred to one end stage so
  the ACT engine stays on a single function table during the main loop.
  Depth extrusion = K=1 outer-product matmul with the depth mask.

Each core writes out[b, d, its 12 rows] = [4, 96, 1152]; host concatenates.
"""

import numpy as np

VOX = 96
SHARP = 100.0
EPS = 1e-8
NCORES = 8
M = VOX * VOX
MP = M // NCORES          # 1152 points per core
CHUNKS = MP // 128        # 9
ROWS = MP // VOX          # 12 grid rows per core
PEDGES = 32               # edges per polygon
BIGD = 1e3                # far-outside distance for dummy (empty-batch) polys

# The affine tables/features are built from bf16-exact split components
# (hi+lo), so plain bf16 matmuls with K=8 reconstruct fp32-grade products at
# full PE speed.
MM_DTYPE = "bfloat16"


def _host_prep(polygons, attributes, validity_scores):
    B, N, P, _ = polygons.shape
    assert P == PEDGES
    valid = np.asarray(validity_scores) >= 0.5
    counts = [max(1, int(v.sum())) for v in valid]   # >=1: empty batch gets a dummy
    offs = np.cumsum([0] + counts)
    NPT = int(offs[-1])
    E = NPT * P

    v0 = np.asarray(polygons, np.float32).astype(np.float64)
    v1 = np.roll(v0, -1, axis=2)
    x0, y0 = v0[..., 0], v0[..., 1]
    x1, y1 = v1[..., 0], v1[..., 1]
    ex, ey = x1 - x0, y1 - y0
    esq = ex * ex + ey * ey
    esq_c = np.maximum(esq, 1e-12)
    Sp = np.sqrt(esq + EPS)
    rt = np.sqrt(esq_c)
    s = ex / (ey + EPS)

    cu = np.stack([ex / Sp, ey / Sp, -(x0 * ex + y0 * ey) / Sp], -1)
    cl = np.stack([-ey / rt, ex / rt, (ey * x0 - ex * y0) / rt], -1)
    cg = np.stack([-np.ones_like(s), s, x0 - s * y0], -1)

    wu = np.zeros((3, E), np.float32)
    wl = np.zeros((3, E), np.float32)
    wg = np.zeros((3, E), np.float32)
    y0r = np.full(E, 5.0, np.float32)
    y1r = np.full(E, 5.0, np.float32)
    sr = np.ones(E, np.float32)
    wl[2, :] = BIGD          # dummy cols: u=0, l=BIGD, G=-1 -> far outside
    wg[2, :] = -1.0

    for b in range(B):
        idx = np.nonzero(valid[b])[0]
        for k, n in enumerate(idx):
            c0 = (offs[b] + k) * P
            sl = slice(c0, c0 + P)
            wu[:, sl] = cu[b, n].T.astype(np.float32)
            wl[:, sl] = cl[b, n].T.astype(np.float32)
            wg[:, sl] = cg[b, n].T.astype(np.float32)
            y0r[sl] = y0[b, n].astype(np.float32)
            y1r[sl] = y1[b, n].astype(np.float32)
            sr[sl] = Sp[b, n].astype(np.float32)

    # split-precision expansion: value = hi + lo with bf16-exact components, so
    # fp32r products are computed exactly by the PE regardless of its internal
    # mantissa truncation (>= 8 bits).  features [hx, lx, hx, hy, ly, hy, 1, 1]
    # against weights [w_hi, w_hi, w_lo, ...] reconstruct px*w to ~2^-16.
    import ml_dtypes

    def b16split(x):
        hi = x.astype(ml_dtypes.bfloat16).astype(np.float32)
        lo = (x - hi).astype(ml_dtypes.bfloat16).astype(np.float32)
        return hi, lo

    def expand_w(w):
        out = np.zeros((8, w.shape[1]), np.float32)
        for i in range(3):
            hi, lo = b16split(w[i])
            j = i * 3 if i < 2 else 6
            if i < 2:
                out[j] = hi; out[j + 1] = hi; out[j + 2] = lo
            else:
                out[6] = hi; out[7] = lo
        return out

    wu = expand_w(wu).astype(ml_dtypes.bfloat16)
    wl = expand_w(wl).astype(ml_dtypes.bfloat16)
    wg = expand_w(wg).astype(ml_dtypes.bfloat16)

    ygrid, xgrid = np.meshgrid(np.arange(VOX, dtype=np.float32),
                               np.arange(VOX, dtype=np.float32), indexing="ij")
    px = (xgrid.ravel() / np.float32(VOX - 1)).astype(np.float32)
    py = (ygrid.ravel() / np.float32(VOX - 1)).astype(np.float32)

    feats, ysqbs, cntbs = [], [], []
    for k in range(NCORES):
        sl = slice(k * MP, (k + 1) * MP)
        hx, lx = b16split(px[sl])
        hy, ly = b16split(py[sl])
        one = np.ones(MP, np.float32)
        f = np.stack([hx, lx, hx, hy, ly, hy, one, one], 0)
        feats.append(np.ascontiguousarray(f.astype(ml_dtypes.bfloat16)))
        # ysq[row, e] = (y0<=py) xor (y1<=py), per grid row of this core,
        # expanded to the per-chunk partition->row broadcast pattern
        rows = (np.arange(ROWS, dtype=np.float32) + k * ROWS) / np.float32(VOX - 1)
        t0c = (y0r[None, :] <= rows[:, None])
        t1c = (y1r[None, :] <= rows[:, None])
        ysq12 = (t0c ^ t1c).astype(np.float32)            # [ROWS, E]
        rowidx = (np.arange(MP) // VOX).astype(np.int64)  # local row per point
        ysqbs.append(np.ascontiguousarray(
            ysq12[rowidx].reshape(CHUNKS, 128, E).astype(ml_dtypes.bfloat16)))
        # per-(row, poly) active-edge counts: cnt = sum ysq*sign(G) + cntb
        cb12 = ysq12.reshape(ROWS, NPT, PEDGES).sum(2)    # [ROWS, NPT]
        cntbs.append(np.ascontiguousarray(
            cb12[rowidx].reshape(CHUNKS, 128, NPT).astype(np.int32)))

    attr = np.asarray(attributes, np.float32)
    norm_h = np.clip(attr[:, 0], 0.0, 1.0)
    hv = np.clip(np.round(norm_h * VOX), 1.0, float(VOX)).astype(np.float32)
    hvs = [0 if not valid[b].any() else int(hv[b]) for b in range(B)]

    tables = {
        "wu": wu, "wl": wl, "wg": wg,
        "sbc": np.ascontiguousarray(np.ones((128, 1), np.float32) * sr[None, :]),
        "ident": np.eye(128, dtype=np.float32),
    }
    return tables, feats, ysqbs, cntbs, counts, E, hvs


def _blocks(E):
    nblk = (E + 511) // 512
    per = -(-E // (32 * nblk)) * 32           # even-ish blocks, multiple of 32
    out = []
    o = 0
    while o < E:
        nb = min(per, E - o)
        out.append((o, nb))
        o += nb
    return out


def _build(B, counts, E, hvs):
    import concourse.tile as tile
    from concourse import bacc, mybir

    f32 = mybir.dt.float32
    i32 = mybir.dt.int32
    bf16 = mybir.dt.bfloat16
    mmdt = getattr(mybir.dt, MM_DTYPE)

    Op = mybir.AluOpType
    Act = mybir.ActivationFunctionType
    X = mybir.AxisListType.X
    NPT = sum(counts)
    offs = np.cumsum([0] + list(counts))
    blocks = _blocks(E)

    nc = bacc.Bacc("TRN2", target_bir_lowering=False, debug=False)

    din = {}
    for name, shape in [("wu", [8, E]), ("wl", [8, E]), ("wg", [8, E]),
                        ("sbc", [128, E]), ("feat", [8, MP]),
                        ("ysqb_all", [CHUNKS, 128, E]),
                        ("cntb_all", [CHUNKS, 128, NPT]),
                        ("ident", [128, 128])]:
        if name in ("wu", "wl", "wg", "feat", "ysqb_all"):
            dt = mmdt
        elif name == "cntb_all":
            dt = mybir.dt.int32
        else:
            dt = f32
        din[name] = nc.dram_tensor(name, shape, dt, kind="ExternalInput")
    out_d = nc.dram_tensor("out", [B, VOX, MP], f32, kind="ExternalOutput")
    comb_d = nc.dram_tensor("comb_scratch", [B, MP], f32)

    with tile.TileContext(nc) as tc:
        with tc.tile_pool(name="const", bufs=1) as cpool, \
             tc.tile_pool(name="work", bufs=4) as wpool, \
             tc.tile_pool(name="ybc", bufs=2) as ypool, \
             tc.tile_pool(name="acc", bufs=2) as apool, \
             tc.tile_pool(name="psum3", bufs=3, space="PSUM") as ppool3, \
             tc.tile_pool(name="psum", bufs=2, space="PSUM") as ppool, \
             tc.tile_pool(name="pout", bufs=1, space="PSUM") as opool:

            sb = {}
            for name in ["wu", "wl", "wg"]:
                t = cpool.tile([8, E], mmdt, tag=f"c_{name}", name=f"c_{name}")
                nc.sync.dma_start(t[:], din[name][:])
                sb[name] = t
            sbc = cpool.tile([128, E], f32)
            for i, (j0, nb) in enumerate(blocks):
                eng = nc.sync if i == 0 else nc.scalar
                eng.dma_start(sbc[:, j0:j0 + nb], din["sbc"][:, j0:j0 + nb])
            feat = cpool.tile([8, MP], mmdt)
            nc.sync.dma_start(feat[:], din["feat"][:])
            ident = cpool.tile([128, 128], f32)
            nc.scalar.dma_start(ident[:], din["ident"][:])
            cntb = cpool.tile([128, CHUNKS, NPT], i32)
            for c in range(CHUNKS):
                nc.scalar.dma_start(cntb[:, c, :], din["cntb_all"][c])

            qall = cpool.tile([128, B * 32], f32)
            nc.gpsimd.memset(qall[:], 0)
            qbig = cpool.tile([128, CHUNKS, NPT], f32)
            comb = []
            for b in range(B):
                comb_b = cpool.tile([CHUNKS, 128], f32, tag=f"comb{b}",
                                    name=f"comb{b}")
                comb.append(comb_b)

            warm = cpool.tile([1, 1], f32)
            nc.gpsimd.memset(warm[:], 0)
            deferred = []

            def run_deferred(keep):
                while len(deferred) > keep:
                    deferred.pop(0)()

            for c in range(CHUNKS):
                featc = feat[:, c * 128:(c + 1) * 128]
                ysqb = ypool.tile([128, E], bf16, tag="ysqb", name="ysqb")
                if c == 0:
                    for i, (j0, nb) in enumerate(blocks):
                        eng = nc.sync if i == 0 else nc.scalar
                        eng.dma_start(ysqb[:, j0:j0 + nb],
                                      din["ysqb_all"][c][:, j0:j0 + nb])
                else:
                    nc.sync.dma_start(ysqb[:], din["ysqb_all"][c])

                mind2 = apool.tile([128, NPT], f32, tag="mind2")
                cnt = apool.tile([128, NPT], i32, tag="cnt")

                def reduces(j0, npj, d2, cross, mind2=None, cnt=None):
                    pj = slice(j0 // PEDGES, j0 // PEDGES + npj)
                    nc.vector.tensor_reduce(
                        mind2[:, pj],
                        d2[:].rearrange("p (a b) -> p a b", b=PEDGES),
                        axis=X, op=Op.min)
                    with nc.allow_low_precision(
                            reason="crossing counts are small exact ints"):
                        nc.vector.tensor_reduce(
                            cnt[:, pj],
                            cross[:].rearrange("p (a b) -> p a b", b=PEDGES),
                            axis=X, op=Op.add)

                for j0, nb in blocks:
                    npj = nb // PEDGES
                    cols = slice(j0, j0 + nb)
                    ups = ppool3.tile([128, nb], f32, tag="u")
                    lps = ppool.tile([128, nb], f32, tag="l")
                    gps = ppool.tile([128, nb], f32, tag="g")
                    nc.tensor.matmul(ups[:], featc, sb["wu"][:, cols])
                    nc.tensor.matmul(lps[:], featc, sb["wl"][:, cols])
                    nc.tensor.matmul(gps[:], featc, sb["wg"][:, cols])

                    m = wpool.tile([128, nb], f32, tag="m")
                    nc.vector.scalar_tensor_tensor(
                        m[:], ups[:], 0.0, sbc[:, cols], op0=Op.max, op1=Op.min)
                    r = wpool.tile([128, nb], f32, tag="r")
                    nc.vector.tensor_tensor(r[:], ups[:], m[:], op=Op.subtract)
                    lsq = wpool.tile([128, nb], f32, tag="lsq")
                    nc.scalar.activation(lsq[:], lps[:], Act.Square)
                    rsq = wpool.tile([128, nb], f32, tag="rsq")
                    nc.scalar.activation(rsq[:], r[:], Act.Square)
                    d2 = wpool.tile([128, nb], f32, tag="d2")
                    nc.gpsimd.tensor_tensor(d2[:], lsq[:], rsq[:], op=Op.add)

                    gs = wpool.tile([128, nb], bf16, tag="gs")
                    nc.scalar.activation(gs[:], gps[:], Act.Sign)
                    cross = wpool.tile([128, nb], bf16, tag="cross")
                    nc.gpsimd.tensor_tensor(cross[:], gs[:], ysqb[:, cols],
                                            op=Op.mult)

                    deferred.append(
                        lambda a=j0, b_=npj, d=d2, x=cross, mi=mind2, cn=cnt:
                        reduces(a, b_, d, x, mi, cn))
                    run_deferred(3)

                def chunk_tail(c=c, mind2=mind2, cnt=cnt):
                    # cnt + cntb = 2*crossings (exact ints); parity from bit 1
                    odd2 = wpool.tile([128, NPT], i32, tag="odd2")
                    cnt2 = wpool.tile([128, NPT], i32, tag="cnt2")
                    nc.vector.tensor_tensor(cnt2[:], cnt[:], cntb[:, c, :],
                                            op=Op.add)
                    nc.vector.tensor_scalar(odd2[:], cnt2[:], 2, None,
                                            op0=Op.bitwise_and)
                    sgn = wpool.tile([128, NPT], f32, tag="sgn")
                    nc.vector.tensor_scalar(sgn[:], odd2[:], -1.0, 1.0,
                                            op0=Op.mult, op1=Op.add)
                    nc.vector.tensor_tensor(qbig[:, c, :], mind2[:], sgn[:],
                                            op=Op.mult)

                deferred.append(chunk_tail)
                if c == CHUNKS - 2:
                    nc.scalar.activation(warm[:], warm[:], Act.Sqrt)
            run_deferred(0)

            # per-batch min over polys, all chunks at once (writes the
            # transpose-ready [128, 32b+c] layout)
            for b in range(B):
                nc.vector.tensor_reduce(
                    qall[:, 32 * b:32 * b + CHUNKS],
                    qbig[:, :, offs[b]:offs[b + 1]], axis=X, op=Op.min)

            # end stage: sdf = sign(q)*sqrt(|q|), one sigmoid + one transpose;
            # after the transpose, batch b's 9 chunk-rows sit at partitions
            # 32b..32b+8 (aligned base for the per-batch copies)
            absq = wpool.tile([128, B * 32], f32, tag="absq")
            nc.scalar.activation(absq[:], qall[:], Act.Abs)
            dst = wpool.tile([128, B * 32], f32, tag="dst")
            nc.scalar.activation(dst[:], absq[:], Act.Sqrt)
            sgq = wpool.tile([128, B * 32], f32, tag="sgq")
            nc.scalar.activation(sgq[:], qall[:], Act.Sign)
            sdf = wpool.tile([128, B * 32], f32, tag="sdf")
            nc.vector.tensor_tensor(sdf[:], dst[:], sgq[:], op=Op.mult)
            cpb = wpool.tile([128, B * 32], f32, tag="cpb")
            nc.scalar.activation(cpb[:], sdf[:], Act.Sigmoid, scale=-SHARP)
            pst = opool.tile([128, 128], f32, tag="pp", name="pst")
            nc.tensor.transpose(pst[:], cpb[:], ident[:])
            for b in range(B):
                nc.scalar.activation(comb[b][:], pst[32 * b:32 * b + CHUNKS, :],
                                     Act.Copy)

            # depth extrusion: replicate combined[b] into rows [0, hv_b) with
            # independent parallel broadcast DMAs (16-row groups) from a DRAM
            # bounce row; rows >= hv_b stay zero (outputs are donated zero
            # buffers).  Dispatch spread over the three DMA-capable engines.
            engs = [nc.sync, nc.gpsimd, nc.scalar]
            ei = 0
            for b in range(B):
                if hvs[b] == 0:
                    continue
                engs[ei % 3].dma_start(comb_d[b:b + 1, :], comb[b][:])
                ei += 1
            GRP = 16
            for b in range(B):
                g0 = 0
                while g0 < hvs[b]:
                    n = min(GRP, hvs[b] - g0)
                    engs[ei % 3].dma_start(
                        out_d[b, g0:g0 + n, :],
                        comb_d[b:b + 1, :].partition_broadcast(n))
                    ei += 1
                    g0 += n

    nc.compile()
    return nc


def kernel(polygons, attributes, validity_scores):
    from concourse.bass_utils import run_bass_kernel_spmd

    B = polygons.shape[0]
    tables, feats, ysqbs, cntbs, counts, E, hvs = _host_prep(
        polygons, attributes, validity_scores)
    nc = _build(B, counts, E, hvs)
    in_maps = [dict(tables, feat=feats[k], ysqb_all=ysqbs[k], cntb_all=cntbs[k])
               for k in range(NCORES)]
    res = run_bass_kernel_spmd(nc, in_maps, list(range(NCORES))).results
    parts = [res[k]["out"].reshape(B, VOX, VOX // NCORES, VOX)
             for k in range(NCORES)]
    return np.ascontiguousarray(np.concatenate(parts, axis=2), np.float32)


# revision 37
# speedup vs baseline: 1.0674x; 1.0052x over previous
"""Trainium2 Bass kernel for DifferentiableExtrusion.

Full inputs in, full output out. Sharding: the 96x96=9216 grid points are
split across 8 cores (12 grid rows / 1152 points each). Every core processes
all valid polygons (host-compacted, variable count per batch) against its
points:

  per (point, edge):  d^2 = l^2 + r^2   with
      l = v . n_hat               (line distance, affine in point -> PE matmul)
      u = v . e / sqrt(e^2+eps)   (affine in point -> PE matmul)
      r = u - clip(u, 0, S)       (projection excess)
  inside test: ray-cast parity of [(sign(py-y0) != sign(py-y1)) & (G > 0)]
      with G = inter_x - px       (affine in point -> PE matmul)
  The y-comparisons depend only on the point's grid row: computed once per
  core at [12, E] and DMA-broadcast across partitions per chunk.
  sdf = sign * sqrt(min_edges d^2); per-batch min over polys taken on
  sign*(d^2) (order-equivalent); sqrt+sigmoid deferred to one end stage so
  the ACT engine stays on a single function table during the main loop.
  Depth extrusion = K=1 outer-product matmul with the depth mask.

Each core writes out[b, d, its 12 rows] = [4, 96, 1152]; host concatenates.
"""

import numpy as np

VOX = 96
SHARP = 100.0
EPS = 1e-8
NCORES = 8
M = VOX * VOX
MP = M // NCORES          # 1152 points per core
CHUNKS = MP // 128        # 9
ROWS = MP // VOX          # 12 grid rows per core
PEDGES = 32               # edges per polygon
BIGD = 1e3                # far-outside distance for dummy (empty-batch) polys

# The affine tables/features are built from bf16-exact split components
# (hi+lo), so plain bf16 matmuls with K=8 reconstruct fp32-grade products at
# full PE speed.
MM_DTYPE = "bfloat16"


def _host_prep(polygons, attributes, validity_scores):
    B, N, P, _ = polygons.shape
    assert P == PEDGES
    valid = np.asarray(validity_scores) >= 0.5
    counts = [max(1, int(v.sum())) for v in valid]   # >=1: empty batch gets a dummy
    offs = np.cumsum([0] + counts)
    NPT = int(offs[-1])
    E = NPT * P

    v0 = np.asarray(polygons, np.float32).astype(np.float64)
    v1 = np.roll(v0, -1, axis=2)
    x0, y0 = v0[..., 0], v0[..., 1]
    x1, y1 = v1[..., 0], v1[..., 1]
    ex, ey = x1 - x0, y1 - y0
    esq = ex * ex + ey * ey
    esq_c = np.maximum(esq, 1e-12)
    Sp = np.sqrt(esq + EPS)
    rt = np.sqrt(esq_c)
    s = ex / (ey + EPS)

    cu = np.stack([ex / Sp, ey / Sp, -(x0 * ex + y0 * ey) / Sp], -1)
    cl = np.stack([-ey / rt, ex / rt, (ey * x0 - ex * y0) / rt], -1)
    cg = np.stack([-np.ones_like(s), s, x0 - s * y0], -1)

    wu = np.zeros((3, E), np.float32)
    wl = np.zeros((3, E), np.float32)
    wg = np.zeros((3, E), np.float32)
    y0r = np.full(E, 5.0, np.float32)
    y1r = np.full(E, 5.0, np.float32)
    sr = np.ones(E, np.float32)
    wl[2, :] = BIGD          # dummy cols: u=0, l=BIGD, G=-1 -> far outside
    wg[2, :] = -1.0

    for b in range(B):
        idx = np.nonzero(valid[b])[0]
        for k, n in enumerate(idx):
            c0 = (offs[b] + k) * P
            sl = slice(c0, c0 + P)
            wu[:, sl] = cu[b, n].T.astype(np.float32)
            wl[:, sl] = cl[b, n].T.astype(np.float32)
            wg[:, sl] = cg[b, n].T.astype(np.float32)
            y0r[sl] = y0[b, n].astype(np.float32)
            y1r[sl] = y1[b, n].astype(np.float32)
            sr[sl] = Sp[b, n].astype(np.float32)

    # split-precision expansion: value = hi + lo with bf16-exact components, so
    # fp32r products are computed exactly by the PE regardless of its internal
    # mantissa truncation (>= 8 bits).  features [hx, lx, hx, hy, ly, hy, 1, 1]
    # against weights [w_hi, w_hi, w_lo, ...] reconstruct px*w to ~2^-16.
    import ml_dtypes

    def b16split(x):
        hi = x.astype(ml_dtypes.bfloat16).astype(np.float32)
        lo = (x - hi).astype(ml_dtypes.bfloat16).astype(np.float32)
        return hi, lo

    def expand_w(w):
        out = np.zeros((8, w.shape[1]), np.float32)
        for i in range(3):
            hi, lo = b16split(w[i])
            j = i * 3 if i < 2 else 6
            if i < 2:
                out[j] = hi; out[j + 1] = hi; out[j + 2] = lo
            else:
                out[6] = hi; out[7] = lo
        return out

    wu = expand_w(wu).astype(ml_dtypes.bfloat16)
    wl = expand_w(wl).astype(ml_dtypes.bfloat16)
    wg = expand_w(wg).astype(ml_dtypes.bfloat16)

    ygrid, xgrid = np.meshgrid(np.arange(VOX, dtype=np.float32),
                               np.arange(VOX, dtype=np.float32), indexing="ij")
    px = (xgrid.ravel() / np.float32(VOX - 1)).astype(np.float32)
    py = (ygrid.ravel() / np.float32(VOX - 1)).astype(np.float32)

    feats, ysqbs, cntbs = [], [], []
    for k in range(NCORES):
        sl = slice(k * MP, (k + 1) * MP)
        hx, lx = b16split(px[sl])
        hy, ly = b16split(py[sl])
        one = np.ones(MP, np.float32)
        f = np.stack([hx, lx, hx, hy, ly, hy, one, one], 0)
        feats.append(np.ascontiguousarray(f.astype(ml_dtypes.bfloat16)))
        # ysq[row, e] = (y0<=py) xor (y1<=py), per grid row of this core,
        # expanded to the per-chunk partition->row broadcast pattern
        rows = (np.arange(ROWS, dtype=np.float32) + k * ROWS) / np.float32(VOX - 1)
        t0c = (y0r[None, :] <= rows[:, None])
        t1c = (y1r[None, :] <= rows[:, None])
        ysq12 = (t0c ^ t1c).astype(np.float32)            # [ROWS, E]
        rowidx = (np.arange(MP) // VOX).astype(np.int64)  # local row per point
        ysqbs.append(np.ascontiguousarray(
            ysq12[rowidx].reshape(CHUNKS, 128, E).astype(ml_dtypes.bfloat16)))
        # per-(row, poly) active-edge counts: cnt = sum ysq*sign(G) + cntb
        cb12 = ysq12.reshape(ROWS, NPT, PEDGES).sum(2)    # [ROWS, NPT]
        cntbs.append(np.ascontiguousarray(
            cb12[rowidx].reshape(CHUNKS, 128, NPT).astype(np.int32)))

    attr = np.asarray(attributes, np.float32)
    norm_h = np.clip(attr[:, 0], 0.0, 1.0)
    hv = np.clip(np.round(norm_h * VOX), 1.0, float(VOX)).astype(np.float32)
    hvs = [0 if not valid[b].any() else int(hv[b]) for b in range(B)]

    tables = {
        "wu": wu, "wl": wl, "wg": wg,
        "sbc": np.ascontiguousarray(np.ones((128, 1), np.float32) * sr[None, :]),
        "ident": np.eye(128, dtype=np.float32),
    }
    return tables, feats, ysqbs, cntbs, counts, E, hvs


def _blocks(E):
    nblk = (E + 511) // 512
    per = -(-E // (32 * nblk)) * 32           # even-ish blocks, multiple of 32
    out = []
    o = 0
    while o < E:
        nb = min(per, E - o)
        out.append((o, nb))
        o += nb
    return out


def _build(B, counts, E, hvs):
    import concourse.tile as tile
    from concourse import bacc, mybir

    f32 = mybir.dt.float32
    i32 = mybir.dt.int32
    bf16 = mybir.dt.bfloat16
    mmdt = getattr(mybir.dt, MM_DTYPE)

    Op = mybir.AluOpType
    Act = mybir.ActivationFunctionType
    X = mybir.AxisListType.X
    NPT = sum(counts)
    offs = np.cumsum([0] + list(counts))
    blocks = _blocks(E)

    nc = bacc.Bacc("TRN2", target_bir_lowering=False, debug=False)

    din = {}
    for name, shape in [("wu", [8, E]), ("wl", [8, E]), ("wg", [8, E]),
                        ("sbc", [128, E]), ("feat", [8, MP]),
                        ("ysqb_all", [CHUNKS, 128, E]),
                        ("cntb_all", [CHUNKS, 128, NPT]),
                        ("ident", [128, 128])]:
        if name in ("wu", "wl", "wg", "feat", "ysqb_all"):
            dt = mmdt
        elif name == "cntb_all":
            dt = mybir.dt.int32
        else:
            dt = f32
        din[name] = nc.dram_tensor(name, shape, dt, kind="ExternalInput")
    out_d = nc.dram_tensor("out", [B, VOX, MP], f32, kind="ExternalOutput")
    comb_d = nc.dram_tensor("comb_scratch", [B, MP], f32)

    with tile.TileContext(nc) as tc:
        with tc.tile_pool(name="const", bufs=1) as cpool, \
             tc.tile_pool(name="work", bufs=4) as wpool, \
             tc.tile_pool(name="ybc", bufs=2) as ypool, \
             tc.tile_pool(name="acc", bufs=2) as apool, \
             tc.tile_pool(name="psum3", bufs=3, space="PSUM") as ppool3, \
             tc.tile_pool(name="psum", bufs=2, space="PSUM") as ppool, \
             tc.tile_pool(name="pout", bufs=1, space="PSUM") as opool:

            feat = cpool.tile([8, MP], mmdt)
            nc.sync.dma_start(feat[:], din["feat"][:])
            sb = {}
            for name in ["wu", "wl", "wg"]:
                t = cpool.tile([8, E], mmdt, tag=f"c_{name}", name=f"c_{name}")
                nc.sync.dma_start(t[:], din[name][:])
                sb[name] = t
            sbc = cpool.tile([128, E], f32)
            for i, (j0, nb) in enumerate(blocks):
                eng = nc.sync if i == 0 else nc.scalar
                eng.dma_start(sbc[:, j0:j0 + nb], din["sbc"][:, j0:j0 + nb])
            ident = cpool.tile([128, 128], f32)
            nc.scalar.dma_start(ident[:], din["ident"][:])
            cntb = cpool.tile([128, CHUNKS, NPT], i32)
            for c in range(CHUNKS):
                nc.scalar.dma_start(cntb[:, c, :], din["cntb_all"][c])

            qall = cpool.tile([128, B * 32], f32)
            nc.gpsimd.memset(qall[:], 0)
            qbig = cpool.tile([128, CHUNKS, NPT], f32)
            comb = []
            for b in range(B):
                comb_b = cpool.tile([CHUNKS, 128], f32, tag=f"comb{b}",
                                    name=f"comb{b}")
                comb.append(comb_b)

            warm = cpool.tile([1, 1], f32)
            nc.gpsimd.memset(warm[:], 0)
            deferred = []

            def run_deferred(keep):
                while len(deferred) > keep:
                    deferred.pop(0)()

            for c in range(CHUNKS):
                featc = feat[:, c * 128:(c + 1) * 128]
                ysqb = ypool.tile([128, E], bf16, tag="ysqb", name="ysqb")
                if c == 0:
                    for i, (j0, nb) in enumerate(blocks):
                        eng = nc.sync if i == 0 else nc.scalar
                        eng.dma_start(ysqb[:, j0:j0 + nb],
                                      din["ysqb_all"][c][:, j0:j0 + nb])
                else:
                    nc.sync.dma_start(ysqb[:], din["ysqb_all"][c])

                mind2 = apool.tile([128, NPT], f32, tag="mind2")
                cnt = apool.tile([128, NPT], i32, tag="cnt")

                def reduces(j0, npj, d2, cross, mind2=None, cnt=None):
                    pj = slice(j0 // PEDGES, j0 // PEDGES + npj)
                    nc.vector.tensor_reduce(
                        mind2[:, pj],
                        d2[:].rearrange("p (a b) -> p a b", b=PEDGES),
                        axis=X, op=Op.min)
                    with nc.allow_low_precision(
                            reason="crossing counts are small exact ints"):
                        nc.vector.tensor_reduce(
                            cnt[:, pj],
                            cross[:].rearrange("p (a b) -> p a b", b=PEDGES),
                            axis=X, op=Op.add)

                for j0, nb in blocks:
                    npj = nb // PEDGES
                    cols = slice(j0, j0 + nb)
                    ups = ppool3.tile([128, nb], f32, tag="u")
                    lps = ppool.tile([128, nb], f32, tag="l")
                    gps = ppool.tile([128, nb], f32, tag="g")
                    nc.tensor.matmul(ups[:], featc, sb["wu"][:, cols])
                    nc.tensor.matmul(lps[:], featc, sb["wl"][:, cols])
                    nc.tensor.matmul(gps[:], featc, sb["wg"][:, cols])

                    m = wpool.tile([128, nb], f32, tag="m")
                    nc.vector.scalar_tensor_tensor(
                        m[:], ups[:], 0.0, sbc[:, cols], op0=Op.max, op1=Op.min)
                    r = wpool.tile([128, nb], f32, tag="r")
                    nc.vector.tensor_tensor(r[:], ups[:], m[:], op=Op.subtract)
                    lsq = wpool.tile([128, nb], f32, tag="lsq")
                    nc.scalar.activation(lsq[:], lps[:], Act.Square)
                    rsq = wpool.tile([128, nb], f32, tag="rsq")
                    nc.scalar.activation(rsq[:], r[:], Act.Square)
                    d2 = wpool.tile([128, nb], f32, tag="d2")
                    nc.gpsimd.tensor_tensor(d2[:], lsq[:], rsq[:], op=Op.add)

                    gs = wpool.tile([128, nb], bf16, tag="gs")
                    nc.scalar.activation(gs[:], gps[:], Act.Sign)
                    cross = wpool.tile([128, nb], bf16, tag="cross")
                    nc.gpsimd.tensor_tensor(cross[:], gs[:], ysqb[:, cols],
                                            op=Op.mult)

                    deferred.append(
                        lambda a=j0, b_=npj, d=d2, x=cross, mi=mind2, cn=cnt:
                        reduces(a, b_, d, x, mi, cn))
                    run_deferred(3)

                def chunk_tail(c=c, mind2=mind2, cnt=cnt):
                    # cnt + cntb = 2*crossings (exact ints); parity from bit 1
                    odd2 = wpool.tile([128, NPT], i32, tag="odd2")
                    cnt2 = wpool.tile([128, NPT], i32, tag="cnt2")
                    nc.vector.tensor_tensor(cnt2[:], cnt[:], cntb[:, c, :],
                                            op=Op.add)
                    nc.vector.tensor_scalar(odd2[:], cnt2[:], 2, None,
                                            op0=Op.bitwise_and)
                    sgn = wpool.tile([128, NPT], f32, tag="sgn")
                    nc.vector.tensor_scalar(sgn[:], odd2[:], -1.0, 1.0,
                                            op0=Op.mult, op1=Op.add)
                    nc.vector.tensor_tensor(qbig[:, c, :], mind2[:], sgn[:],
                                            op=Op.mult)

                deferred.append(chunk_tail)
                if c == CHUNKS - 2:
                    nc.scalar.activation(warm[:], warm[:], Act.Sqrt)
            run_deferred(0)

            # per-batch min over polys, all chunks at once (writes the
            # transpose-ready [128, 32b+c] layout)
            for b in range(B):
                nc.vector.tensor_reduce(
                    qall[:, 32 * b:32 * b + CHUNKS],
                    qbig[:, :, offs[b]:offs[b + 1]], axis=X, op=Op.min)

            # end stage: sdf = sign(q)*sqrt(|q|), one sigmoid + one transpose;
            # after the transpose, batch b's 9 chunk-rows sit at partitions
            # 32b..32b+8 (aligned base for the per-batch copies)
            absq = wpool.tile([128, B * 32], f32, tag="absq")
            nc.scalar.activation(absq[:], qall[:], Act.Abs)
            dst = wpool.tile([128, B * 32], f32, tag="dst")
            nc.scalar.activation(dst[:], absq[:], Act.Sqrt)
            sgq = wpool.tile([128, B * 32], f32, tag="sgq")
            nc.scalar.activation(sgq[:], qall[:], Act.Sign)
            sdf = wpool.tile([128, B * 32], f32, tag="sdf")
            nc.vector.tensor_tensor(sdf[:], dst[:], sgq[:], op=Op.mult)
            cpb = wpool.tile([128, B * 32], f32, tag="cpb")
            nc.scalar.activation(cpb[:], sdf[:], Act.Sigmoid, scale=-SHARP)
            pst = opool.tile([128, 128], f32, tag="pp", name="pst")
            nc.tensor.transpose(pst[:], cpb[:], ident[:])
            for b in range(B):
                nc.scalar.activation(comb[b][:], pst[32 * b:32 * b + CHUNKS, :],
                                     Act.Copy)

            # depth extrusion: replicate combined[b] into rows [0, hv_b) with
            # independent parallel broadcast DMAs (16-row groups) from a DRAM
            # bounce row; rows >= hv_b stay zero (outputs are donated zero
            # buffers).  Dispatch spread over the three DMA-capable engines.
            engs = [nc.sync, nc.gpsimd, nc.scalar]
            ei = 0
            for b in range(B):
                if hvs[b] == 0:
                    continue
                engs[ei % 3].dma_start(comb_d[b:b + 1, :], comb[b][:])
                ei += 1
            GRP = 16
            for b in range(B):
                g0 = 0
                while g0 < hvs[b]:
                    n = min(GRP, hvs[b] - g0)
                    engs[ei % 3].dma_start(
                        out_d[b, g0:g0 + n, :],
                        comb_d[b:b + 1, :].partition_broadcast(n))
                    ei += 1
                    g0 += n

    nc.compile()
    return nc


def kernel(polygons, attributes, validity_scores):
    from concourse.bass_utils import run_bass_kernel_spmd

    B = polygons.shape[0]
    tables, feats, ysqbs, cntbs, counts, E, hvs = _host_prep(
        polygons, attributes, validity_scores)
    nc = _build(B, counts, E, hvs)
    in_maps = [dict(tables, feat=feats[k], ysqb_all=ysqbs[k], cntb_all=cntbs[k])
               for k in range(NCORES)]
    res = run_bass_kernel_spmd(nc, in_maps, list(range(NCORES))).results
    parts = [res[k]["out"].reshape(B, VOX, VOX // NCORES, VOX)
             for k in range(NCORES)]
    return np.ascontiguousarray(np.concatenate(parts, axis=2), np.float32)
